# revision 1
# baseline (speedup 1.0000x reference)
"""Cross-attention Trainium2 kernel (8 NeuronCores, SPMD, no collectives).

Sharding: core c -> batch b = c//2, head-group g = c%2 (8 of 16 heads).
Q/K/V projections are column-parallel over the head group's 512 features,
fc_out is row-parallel; the two per-batch partial outputs are summed on the
host during the gather step (replacing the all-reduce).

Per-core dataflow (feature-major / transposed so the contraction dim always
lands on SBUF partitions; host supplies X.T cast to bf16; PSUM accumulation
and softmax internals stay fp32):
  A) V token-major [k, f] with a ones column per head (softmax denom);
     Q.T/K.T feature-major [f, s] per head pair.
  B) S.T[k_chunk, q] = K.T-chunk.T @ Q.T -- the two heads of a pair issue
     back-to-back into PE row groups 0-63/64-127 (concurrent on HW).
  C) attn.T = exp(S.T/8) on ScalarE, [128,1024] batched PSUM reads.
  D) O'.T[65, q] = sum_k V'[k, 65].T @ attn.T[k, q] (row 64 = denominator),
     lagged 2 kc-pairs behind B' so ScalarE never starves.
  E) normalize O.T rows by broadcast reciprocal denominator (+ bv).
  F) F.T[e, s] = sum_g Wo[g, e].T @ O.T[g, s] -> DMA transposed partial.
Host gather: out[b] = (F.T(core 2b) + F.T(core 2b+1)).T + bo.

The schedule is an explicit software pipeline over items (head-pair, qc):
ScalarE exp (~18.3us/item) is the pacer; V-projection, next-pair Q/K
projections, and the first output-projection half are sprinkled into PE
slack slots inside each item.

Softmax uses no max-subtraction: exp runs in fp32 from PSUM scores, safe
for |score| < ~80 (scores ~ N(0,1) here).
"""

import math

import numpy as np

import concourse.bass as bass
import concourse.mybir as mybir
import concourse.tile as tile
from concourse import bacc
from concourse.bass_utils import run_bass_kernel_spmd

# Problem dims (hardcoded per contract).
B = 4
S_Q = 1024      # decoder tokens
S_K = 2048      # encoder tokens
D = 1024        # embed dim
DK = 64         # head dim
H = 8           # heads per core (16 total / 2 groups)
F = H * DK      # 512 per-core head features
P = 128
NKC = S_K // P  # 16 k-chunks
QC = 512        # q tile (matmul moving dim)
NQC = S_Q // QC
SCALE = 1.0 / math.sqrt(DK)

F32 = mybir.dt.float32

# Matmul compute dtype: "f32" (exact, 4 cyc/row) or "bf16" (1 cyc/row,
# inputs host-cast to bfloat16, fp32 PSUM accumulation and fp32 softmax).
MM_MODE = "bf16"
MDT = mybir.dt.bfloat16 if MM_MODE == "bf16" else mybir.dt.float32

_CACHE = {}
SKIP_DMA = False   # bench ablation: drop all DMA traffic
VP_CONTIG = False  # use 8 contiguous VP copies instead of one strided copy
MM_BCAST = False   # broadcast reciprocal denom via K=1 matmul instead of GpSimd
NO_PACK = False    # emit B' head-pair matmuls non-adjacent (disable row packing)


def build_program(repeat=1):
    nc = bacc.Bacc("TRN2", target_bir_lowering=False)

    xqT = nc.dram_tensor("xqT", [D, S_Q], MDT, kind="ExternalInput")
    xkT = nc.dram_tensor("xkT", [D, S_K], MDT, kind="ExternalInput")
    xvT = nc.dram_tensor("xvT", [D, S_K], MDT, kind="ExternalInput")
    wq = nc.dram_tensor("wq", [D, F], MDT, kind="ExternalInput")
    wk = nc.dram_tensor("wk", [D, F], MDT, kind="ExternalInput")
    wv = nc.dram_tensor("wv", [D, F], MDT, kind="ExternalInput")
    wo = nc.dram_tensor("wo", [F, D], MDT, kind="ExternalInput")
    bq = nc.dram_tensor("bq", [F], F32, kind="ExternalInput")
    bk = nc.dram_tensor("bk", [F], F32, kind="ExternalInput")
    bv = nc.dram_tensor("bv", [F], F32, kind="ExternalInput")
    fT = nc.dram_tensor("fT", [D, S_Q], F32, kind="ExternalOutput")

    ADD = mybir.AluOpType.add
    MUL = mybir.AluOpType.mult
    EXP = mybir.ActivationFunctionType.Exp

    with tile.TileContext(nc) as tc:
        with (
            tc.tile_pool(name="const", bufs=1) as cpool,
            tc.tile_pool(name="wt", bufs=2) as wpool,
            tc.tile_pool(name="xtv", bufs=3) as xtvpool,
            tc.tile_pool(name="slab", bufs=3) as apool,
            tc.tile_pool(name="small", bufs=2) as spool,
            tc.tile_pool(name="fo", bufs=3) as fopool,
            tc.tile_pool(name="psum", bufs=1, space="PSUM") as psum,
        ):
            # Persistent tensors. QT doubles as O.T storage after each head's
            # attention output is normalized (write-after-read, disjoint rows
            # per head; Tile tracks the hazard).
            def emit_all():
                _emit(nc, tc, cpool, wpool, xtvpool, apool, spool, fopool, psum,
                      xqT, xkT, xvT, wq, wk, wv, wo, bq, bk, bv, fT)

            if repeat == 1:
                emit_all()
            else:
                with tc.For_i(0, repeat, 1):
                    emit_all()

    nc.finalize()
    return nc


def _emit(nc, tc, cpool, wpool, xtvpool, apool, spool, fopool, psum,
          xqT, xkT, xvT, wq, wk, wv, wo, bq, bk, bv, fT):
    def dma(out, in_):
        if not SKIP_DMA:
            nc.sync.dma_start(out, in_)
    ADD = mybir.AluOpType.add
    MUL = mybir.AluOpType.mult
    EXP = mybir.ActivationFunctionType.Exp
    if True:
        if True:
            XQ = cpool.tile([P, 8, S_Q], MDT)
            XK = cpool.tile([P, 8, S_K], MDT)
            QT = cpool.tile([P, 4, S_Q], MDT)   # [p, head-pair, s]
            KT = cpool.tile([P, 4, S_K], MDT)
            VP = cpool.tile([P, NKC, H * (DK + 1)], MDT)       # 65 cols per head
            WO = cpool.tile([P, 4, D], MDT)
            BIAS = cpool.tile([P, 12], F32)                    # bq|bk|bv as [128,4]
            ONES1 = cpool.tile([1, DK], F32)
            nc.vector.memset(ONES1[:], 1.0)

            # --- loads, ordered by first use ---
            wt_v = wpool.tile([P, 8, F], MDT, tag="wv", name="wt_v")
            # split so the first V-projection matmuls (dc 0-3) start sooner
            wv_r = wv.rearrange("(o p) f -> p o f", p=P)
            dma(wt_v[:, 0:4, :], wv_r[:, 0:4, :])
            dma(wt_v[:, 4:8, :], wv_r[:, 4:8, :])
            ones_view = VP.rearrange("p c (h x) -> p c h x", x=DK + 1)[:, :, :, DK:]
            nc.vector.memset(ones_view, 1.0)
            dma(BIAS[:, 0:4], bq.rearrange("(o p) -> p o", p=P))
            dma(BIAS[:, 4:8], bk.rearrange("(o p) -> p o", p=P))
            dma(BIAS[:, 8:12], bv.rearrange("(o p) -> p o", p=P))

            def v_unit(kc, xtv_tile, j):
                """V projection for one k-chunk (token-major into VP)."""
                ps = psum.tile([P, F], F32, tag="a", bufs=2, name="ps_av")
                for dc in range(8):
                    nc.tensor.matmul(
                        ps[:],
                        xtv_tile[:, dc, j * P:(j + 1) * P],
                        wt_v[:, dc, :],
                        start=(dc == 0),
                        stop=(dc == 7),
                    )
                if VP_CONTIG:
                    for h in range(H):
                        nc.vector.tensor_copy(
                            VP[:, kc, h * (DK + 1):h * (DK + 1) + DK],
                            ps[:, h * DK:(h + 1) * DK],
                        )
                else:
                    nc.vector.tensor_copy(
                        VP[:, kc].rearrange("p (h x) -> p h x", x=DK + 1)[:, :, 0:DK],
                        ps.rearrange("p (h x) -> p h x", x=DK),
                    )

            def v_chunk_units(kc4):
                """Two sprinkle units sharing one DMA'd xvT tile."""
                state = {}

                def unit0():
                    xtv = xtvpool.tile([P, 8, 2 * P], MDT, tag="xtv", name="xtv")
                    xv_r = xvT[:, kc4 * 2 * P:(kc4 + 1) * 2 * P].rearrange(
                        "(o p) k -> p o k", p=P
                    )
                    dma(xtv[:, 0:4, :], xv_r[:, 0:4, :])
                    dma(xtv[:, 4:8, :], xv_r[:, 4:8, :])
                    state["xtv"] = xtv
                    v_unit(2 * kc4, xtv, 0)

                def unit1():
                    v_unit(2 * kc4 + 1, state["xtv"], 1)

                return [unit0, unit1]

            def proj_unit(x_sb, wt, dest, bias_col, hp, sc):
                """One feature-major projection psum group (8 matmuls)."""
                fsl = slice(hp * P, (hp + 1) * P)
                ps = psum.tile([P, QC], F32, tag="a", bufs=2, name="ps_a")
                for dc in range(8):
                    nc.tensor.matmul(
                        ps[:],
                        wt[:, dc, fsl],
                        x_sb[:, dc, sc * QC:(sc + 1) * QC],
                        start=(dc == 0),
                        stop=(dc == 7),
                    )
                nc.vector.tensor_tensor(
                    dest[:, hp, sc * QC:(sc + 1) * QC],
                    ps[:],
                    BIAS[:, bias_col + hp:bias_col + hp + 1].to_broadcast((P, QC)),
                    ADD,
                )

            def f_unit(sc, ec, use_act=False):
                """One output-projection group: F.T[ec, sc] -> DRAM."""
                ss = slice(sc * QC, (sc + 1) * QC)
                pf = psum.tile([P, QC], F32, tag="a", bufs=2, name="ps_f")
                for gc in range(4):
                    nc.tensor.matmul(
                        pf[:],
                        WO[:, gc, ec * P:(ec + 1) * P],
                        QT[:, gc, ss],
                        start=(gc == 0),
                        stop=(gc == 3),
                    )
                fo = fopool.tile([P, QC], F32, tag="fo", name="fo")
                if use_act:  # ScalarE is idle once the last exp retired
                    nc.scalar.copy(fo[:], pf[:])
                else:
                    nc.vector.tensor_copy(fo[:], pf[:])
                dma(fT[ec * P:(ec + 1) * P, ss], fo[:])

            wt_q = wpool.tile([P, 8, F], MDT, tag="w", name="wt_q")
            wt_k = wpool.tile([P, 8, F], MDT, tag="w", name="wt_k")

            def load_qk():
                dma(wt_q[:], wq.rearrange("(o p) f -> p o f", p=P))
                dma(wt_k[:], wk.rearrange("(o p) f -> p o f", p=P))
                for sc in range(NQC):
                    ssl = slice(sc * QC, (sc + 1) * QC)
                    dma(
                        XQ[:, :, ssl],
                        xqT[:, ssl].rearrange("(o p) s -> p o s", p=P),
                    )
                for sc in range(S_K // QC):
                    ssl = slice(sc * QC, (sc + 1) * QC)
                    dma(
                        XK[:, :, ssl],
                        xkT[:, ssl].rearrange("(o p) s -> p o s", p=P),
                    )

            def normalize(h, qc, po):
                hp, hr = h // 2, DK * (h % 2)
                qs = slice(qc * QC, (qc + 1) * QC)
                r = spool.tile([1, QC], F32, tag="r", name="r")
                nc.vector.reciprocal(r[:], po[DK:DK + 1, :])
                R = spool.tile([DK, QC], F32, tag="R", name="R")
                if MM_BCAST:
                    rps = psum.tile([DK, QC], F32, tag="s", bufs=2, name="rps")
                    nc.tensor.matmul(rps[:], ONES1[:, 0:DK], r[:],
                                     start=True, stop=True)
                    nc.vector.tensor_copy(R[:], rps[:])
                else:
                    nc.gpsimd.partition_broadcast(R[:], r[:])
                dest = QT[hr:hr + DK, hp, qs]
                nc.vector.tensor_tensor(dest, po[0:DK, :], R[:], MUL)
                nc.vector.tensor_tensor(
                    dest,
                    dest,
                    BIAS[hr:hr + DK, 8 + hp:9 + hp].to_broadcast((DK, QC)),
                    ADD,
                )

            def pair_item(hp, qc, sprinkles):
                """Scores+exp+attend for heads (2hp, 2hp+1) at q-chunk qc.

                B' kc-pairs feed ScalarE; D lags 2 kc-pairs behind; sprinkle
                units fill remaining PE slack.
                """
                qs = slice(qc * QC, (qc + 1) * QC)
                hA, hB = 2 * hp, 2 * hp + 1
                slabs = {}
                pos = {}
                for h in (hA, hB):
                    slabs[h] = apool.tile([P, NKC, QC], MDT, tag="slab",
                                          name="slab")
                    pos[h] = psum.tile([DK + 1, QC], F32, tag="o", bufs=2,
                                       name="ps_o")

                def d_pair(kc2):
                    for h in (hA, hB):
                        for kc in (2 * kc2, 2 * kc2 + 1):
                            nc.tensor.matmul(
                                pos[h][:],
                                VP[:, kc, h * (DK + 1):(h + 1) * (DK + 1)],
                                slabs[h][:, kc, :],
                                start=(kc == 0),
                                stop=(kc == NKC - 1),
                            )

                spr = list(sprinkles)
                for kc2 in range(NKC // 2):
                    ps2 = {}
                    for h in (hA, hB):
                        ps2[h] = psum.tile([P, 2 * QC], F32, tag="s", bufs=2,
                                           name="ps_s")
                    # the pair's matmuls issue back-to-back per kc so the two
                    # PE row groups (partitions 0-63 / 64-127) overlap
                    if NO_PACK:
                        emit_order = [(j, h) for h in (hA, hB) for j in range(2)]
                    else:
                        emit_order = [(j, h) for j in range(2) for h in (hA, hB)]
                    for j, h in emit_order:
                        kc = 2 * kc2 + j
                        hr = DK * (h % 2)
                        nc.tensor.matmul(
                            ps2[h][:, j * QC:(j + 1) * QC],
                            KT[hr:hr + DK, hp, kc * P:(kc + 1) * P],
                            QT[hr:hr + DK, hp, qs],
                            start=True,
                            stop=True,
                        )
                    for h in (hA, hB):
                        nc.scalar.activation(
                            slabs[h][:, 2 * kc2:2 * kc2 + 2, :].rearrange(
                                "p a b -> p (a b)"
                            ),
                            ps2[h][:],
                            EXP, scale=SCALE,
                        )
                    if spr:
                        spr.pop(0)()
                    if kc2 >= 2:
                        d_pair(kc2 - 2)
                for s in spr:  # any sprinkles that didn't fit the kc2 slots
                    s()
                d_pair(NKC // 2 - 2)
                d_pair(NKC // 2 - 1)
                for h in (hA, hB):
                    normalize(h, qc, pos[h])

            # ---------------- schedule ----------------
            vu = {kc4: v_chunk_units(kc4) for kc4 in range(8)}
            # preamble: V chunks 0-3, then Q/K load + head-pair-0 projections
            for kc4 in range(4):
                for u in vu[kc4]:
                    u()
            load_qk()
            for sc in range(NQC):
                proj_unit(XQ, wt_q, QT, 0, 0, sc)
            for sc in range(S_K // QC):
                proj_unit(XK, wt_k, KT, 4, 0, sc)

            def wo_unit():
                dma(WO[:], wo.rearrange("(o p) e -> p o e", p=P))

            def projs(hp):
                return (
                    [lambda sc=sc, hp=hp: proj_unit(XQ, wt_q, QT, 0, hp, sc)
                     for sc in range(NQC)]
                    + [lambda sc=sc, hp=hp: proj_unit(XK, wt_k, KT, 4, hp, sc)
                       for sc in range(S_K // QC)]
                )

            sprinkle_plan = {
                0: vu[4] + vu[5] + vu[6] + vu[7],      # (0,0): V chunks 4-7
                1: projs(1) + [wo_unit],               # (0,1)
                2: projs(2)[:3],                       # (1,0)
                3: projs(2)[3:],                       # (1,1)
                4: projs(3)[:3],                       # (2,0)
                5: projs(3)[3:],                       # (2,1)
                6: [],                                 # (3,0)
                7: [lambda ec=ec: f_unit(0, ec) for ec in range(8)],  # (3,1)
            }

            items = [(hp, qc) for hp in range(4) for qc in range(NQC)]

            for i, (hp, qc) in enumerate(items):
                pair_item(hp, qc, sprinkle_plan[i])
            for ec in range(8):
                f_unit(1, ec)


def _get_program():
    if "nc" not in _CACHE:
        _CACHE["nc"] = build_program()
    return _CACHE["nc"]


def make_in_maps(Q_decoder, K_encoder, V_encoder, Wq, bq, Wk, bk, Wv, bv, Wo):
    if MM_MODE == "bf16":
        import ml_dtypes
        mdt = np.dtype(ml_dtypes.bfloat16)
    else:
        mdt = np.dtype(np.float32)

    def mcast(x):
        return np.ascontiguousarray(np.asarray(x, dtype=np.float32).astype(mdt))

    def f32(x):
        return np.ascontiguousarray(np.asarray(x, dtype=np.float32))

    xT = {
        "xqT": [mcast(np.asarray(Q_decoder[b], np.float32).T) for b in range(B)],
        "xkT": [mcast(np.asarray(K_encoder[b], np.float32).T) for b in range(B)],
        "xvT": [mcast(np.asarray(V_encoder[b], np.float32).T) for b in range(B)],
    }
    Wq, Wk, Wv, Wo = (np.asarray(w, np.float32) for w in (Wq, Wk, Wv, Wo))
    bq, bk, bv = (np.asarray(v, np.float32) for v in (bq, bk, bv))

    in_maps = []
    for c in range(8):
        b, g = c // 2, c % 2
        cols = slice(F * g, F * (g + 1))
        in_maps.append({
            "xqT": xT["xqT"][b],
            "xkT": xT["xkT"][b],
            "xvT": xT["xvT"][b],
            "wq": mcast(Wq[:, cols]),
            "wk": mcast(Wk[:, cols]),
            "wv": mcast(Wv[:, cols]),
            "wo": mcast(Wo[cols, :]),
            "bq": f32(bq[cols]),
            "bk": f32(bk[cols]),
            "bv": f32(bv[cols]),
        })
    return in_maps


def gather(results, bo):
    bo = np.asarray(bo, dtype=np.float32)
    out = np.empty((B, S_Q, D), dtype=np.float32)
    for b in range(B):
        acc = results[2 * b]["fT"] + results[2 * b + 1]["fT"]
        out[b] = acc.T + bo
    return out


def kernel(**inputs) -> np.ndarray:
    nc = _get_program()
    in_maps = make_in_maps(**{k: v for k, v in inputs.items() if k != "bo"})
    res = run_bass_kernel_spmd(nc, in_maps, core_ids=list(range(8)))
    _CACHE["last_results"] = res
    return gather(res.results, inputs["bo"])



# revision 12
# speedup vs baseline: 1.1806x; 1.1806x over previous
"""Cross-attention Trainium2 kernel (8 NeuronCores, SPMD, no collectives).

Sharding: core c -> batch b = c//2, head-group g = c%2 (8 of 16 heads).
Q/K/V projections are column-parallel over the head group's 512 features,
fc_out is row-parallel; the two per-batch partial outputs are summed on the
host during the gather step (replacing the all-reduce).

Per-core dataflow (feature-major / transposed so the contraction dim always
lands on SBUF partitions; host supplies X.T cast to bf16; PSUM accumulation
and softmax internals stay fp32):
  A) V token-major [k, f] with a ones column per head (softmax denom);
     Q.T/K.T feature-major [f, s] per head pair.
  B) S.T[k_chunk, q] = K.T-chunk.T @ Q.T -- the two heads of a pair issue
     back-to-back into PE row groups 0-63/64-127 (concurrent on HW).
  C) attn.T = exp(S.T/8) on ScalarE, [128,1024] batched PSUM reads.
  D) O'.T[65, q] = sum_k V'[k, 65].T @ attn.T[k, q] (row 64 = denominator),
     lagged 2 kc-pairs behind B' so ScalarE never starves.
  E) normalize O.T rows by broadcast reciprocal denominator (+ bv).
  F) F.T[e, s] = sum_g Wo[g, e].T @ O.T[g, s] -> DMA transposed partial.
Host gather: out[b] = (F.T(core 2b) + F.T(core 2b+1)).T + bo.

The schedule is an explicit software pipeline over items (head-pair, qc):
ScalarE exp (~18.3us/item) is the pacer; V-projection, next-pair Q/K
projections, and the first output-projection half are sprinkled into PE
slack slots inside each item.

Softmax uses no max-subtraction: exp runs in fp32 from PSUM scores, safe
for |score| < ~80 (scores ~ N(0,1) here).
"""

import math

import numpy as np

import concourse.bass as bass
import concourse.mybir as mybir
import concourse.tile as tile
from concourse import bacc
from concourse.bass_utils import run_bass_kernel_spmd

# Problem dims (hardcoded per contract).
B = 4
S_Q = 1024      # decoder tokens
S_K = 2048      # encoder tokens
D = 1024        # embed dim
DK = 64         # head dim
H = 8           # heads per core (16 total / 2 groups)
F = H * DK      # 512 per-core head features
P = 128
NKC = S_K // P  # 16 k-chunks
QC = 512        # q tile (matmul moving dim)
NQC = S_Q // QC
SCALE = 1.0 / math.sqrt(DK)

F32 = mybir.dt.float32

# Matmul compute dtype: "f32" (exact, 4 cyc/row) or "bf16" (1 cyc/row,
# inputs host-cast to bfloat16, fp32 PSUM accumulation and fp32 softmax).
MM_MODE = "bf16"
MDT = mybir.dt.bfloat16 if MM_MODE == "bf16" else mybir.dt.float32

_CACHE = {}
SKIP_DMA = False   # bench ablation: drop all DMA traffic
VP_CONTIG = False  # use 8 contiguous VP copies instead of one strided copy
MM_BCAST = False   # broadcast reciprocal denom via K=1 matmul instead of GpSimd
NO_PACK = False    # emit B' head-pair matmuls non-adjacent (disable row packing)


def build_program(repeat=1):
    nc = bacc.Bacc("TRN2", target_bir_lowering=False)

    xqT = nc.dram_tensor("xqT", [D, S_Q], MDT, kind="ExternalInput")
    xkT = nc.dram_tensor("xkT", [D, S_K], MDT, kind="ExternalInput")
    xvT = nc.dram_tensor("xvT", [D, S_K], MDT, kind="ExternalInput")
    wq = nc.dram_tensor("wq", [D, F], MDT, kind="ExternalInput")
    wk = nc.dram_tensor("wk", [D, F], MDT, kind="ExternalInput")
    wv = nc.dram_tensor("wv", [D, F], MDT, kind="ExternalInput")
    wo = nc.dram_tensor("wo", [F, D], MDT, kind="ExternalInput")
    bq = nc.dram_tensor("bq", [F], F32, kind="ExternalInput")
    bk = nc.dram_tensor("bk", [F], F32, kind="ExternalInput")
    bv = nc.dram_tensor("bv", [F], F32, kind="ExternalInput")
    fT = nc.dram_tensor("fT", [D, S_Q], F32, kind="ExternalOutput")

    ADD = mybir.AluOpType.add
    MUL = mybir.AluOpType.mult
    EXP = mybir.ActivationFunctionType.Exp

    with tile.TileContext(nc) as tc:
        with (
            tc.tile_pool(name="const", bufs=1) as cpool,
            tc.tile_pool(name="wt", bufs=2) as wpool,
            tc.tile_pool(name="xtv", bufs=3) as xtvpool,
            tc.tile_pool(name="slab", bufs=3) as apool,
            tc.tile_pool(name="small", bufs=2) as spool,
            tc.tile_pool(name="fo", bufs=3) as fopool,
            tc.tile_pool(name="psum", bufs=1, space="PSUM") as psum,
        ):
            # Persistent tensors. QT doubles as O.T storage after each head's
            # attention output is normalized (write-after-read, disjoint rows
            # per head; Tile tracks the hazard).
            def emit_all():
                _emit(nc, tc, cpool, wpool, xtvpool, apool, spool, fopool, psum,
                      xqT, xkT, xvT, wq, wk, wv, wo, bq, bk, bv, fT)

            if repeat == 1:
                emit_all()
            else:
                with tc.For_i(0, repeat, 1):
                    emit_all()

    nc.finalize()
    return nc


def _emit(nc, tc, cpool, wpool, xtvpool, apool, spool, fopool, psum,
          xqT, xkT, xvT, wq, wk, wv, wo, bq, bk, bv, fT):
    def dma(out, in_):
        if not SKIP_DMA:
            nc.sync.dma_start(out, in_)
    ADD = mybir.AluOpType.add
    MUL = mybir.AluOpType.mult
    EXP = mybir.ActivationFunctionType.Exp
    if True:
        if True:
            XQ = cpool.tile([P, 8, S_Q], MDT)
            XK = cpool.tile([P, 8, S_K], MDT)
            QT = cpool.tile([P, 4, S_Q], MDT)   # [p, head-pair, s]
            KT = cpool.tile([P, 4, S_K], MDT)
            VP = cpool.tile([P, NKC, H * (DK + 1)], MDT)       # 65 cols per head
            WO = cpool.tile([P, 4, D], MDT)
            BIAS = cpool.tile([P, 12], F32)                    # bq|bk|bv as [128,4]
            ONES1 = cpool.tile([1, DK], F32)
            nc.vector.memset(ONES1[:], 1.0)

            # --- loads, ordered by first use ---
            wt_v = wpool.tile([P, 8, F], MDT, tag="wv", name="wt_v")
            # split so the first V-projection matmuls (dc 0-3) start sooner
            wv_r = wv.rearrange("(o p) f -> p o f", p=P)
            dma(wt_v[:, 0:4, :], wv_r[:, 0:4, :])
            dma(wt_v[:, 4:8, :], wv_r[:, 4:8, :])
            ones_view = VP.rearrange("p c (h x) -> p c h x", x=DK + 1)[:, :, :, DK:]
            nc.vector.memset(ones_view, 1.0)
            dma(BIAS[:, 0:4], bq.rearrange("(o p) -> p o", p=P))
            dma(BIAS[:, 4:8], bk.rearrange("(o p) -> p o", p=P))
            dma(BIAS[:, 8:12], bv.rearrange("(o p) -> p o", p=P))

            def v_unit(kc, xtv_tile, j):
                """V projection for one k-chunk (token-major into VP)."""
                ps = psum.tile([P, F], F32, tag="a", bufs=2, name="ps_av")
                for dc in range(8):
                    nc.tensor.matmul(
                        ps[:],
                        xtv_tile[:, dc, j * P:(j + 1) * P],
                        wt_v[:, dc, :],
                        start=(dc == 0),
                        stop=(dc == 7),
                    )
                if VP_CONTIG:
                    for h in range(H):
                        nc.vector.tensor_copy(
                            VP[:, kc, h * (DK + 1):h * (DK + 1) + DK],
                            ps[:, h * DK:(h + 1) * DK],
                        )
                else:
                    nc.vector.tensor_copy(
                        VP[:, kc].rearrange("p (h x) -> p h x", x=DK + 1)[:, :, 0:DK],
                        ps.rearrange("p (h x) -> p h x", x=DK),
                    )

            def v_chunk_units(kc4):
                """Two sprinkle units sharing one DMA'd xvT tile."""
                state = {}

                def unit0():
                    xtv = xtvpool.tile([P, 8, 2 * P], MDT, tag="xtv", name="xtv")
                    xv_r = xvT[:, kc4 * 2 * P:(kc4 + 1) * 2 * P].rearrange(
                        "(o p) k -> p o k", p=P
                    )
                    dma(xtv[:, 0:4, :], xv_r[:, 0:4, :])
                    dma(xtv[:, 4:8, :], xv_r[:, 4:8, :])
                    state["xtv"] = xtv
                    v_unit(2 * kc4, xtv, 0)

                def unit1():
                    v_unit(2 * kc4 + 1, state["xtv"], 1)

                return [unit0, unit1]

            def proj_unit(x_sb, wt, dest, bias_col, hp, sc):
                """One feature-major projection psum group (8 matmuls)."""
                fsl = slice(hp * P, (hp + 1) * P)
                ps = psum.tile([P, QC], F32, tag="a", bufs=2, name="ps_a")
                for dc in range(8):
                    nc.tensor.matmul(
                        ps[:],
                        wt[:, dc, fsl],
                        x_sb[:, dc, sc * QC:(sc + 1) * QC],
                        start=(dc == 0),
                        stop=(dc == 7),
                    )
                nc.vector.tensor_tensor(
                    dest[:, hp, sc * QC:(sc + 1) * QC],
                    ps[:],
                    BIAS[:, bias_col + hp:bias_col + hp + 1].to_broadcast((P, QC)),
                    ADD,
                )

            def f_unit(sc, ec, use_act=False):
                """One output-projection group: F.T[ec, sc] -> DRAM."""
                ss = slice(sc * QC, (sc + 1) * QC)
                pf = psum.tile([P, QC], F32, tag="a", bufs=2, name="ps_f")
                for gc in range(4):
                    nc.tensor.matmul(
                        pf[:],
                        WO[:, gc, ec * P:(ec + 1) * P],
                        QT[:, gc, ss],
                        start=(gc == 0),
                        stop=(gc == 3),
                    )
                fo = fopool.tile([P, QC], F32, tag="fo", name="fo")
                if use_act:  # ScalarE is idle once the last exp retired
                    nc.scalar.copy(fo[:], pf[:])
                else:
                    nc.vector.tensor_copy(fo[:], pf[:])
                dma(fT[ec * P:(ec + 1) * P, ss], fo[:])

            wt_q = wpool.tile([P, 8, F], MDT, tag="w", name="wt_q")
            wt_k = wpool.tile([P, 8, F], MDT, tag="w", name="wt_k")

            def load_qk():
                dma(wt_q[:], wq.rearrange("(o p) f -> p o f", p=P))
                dma(wt_k[:], wk.rearrange("(o p) f -> p o f", p=P))
                for sc in range(NQC):
                    ssl = slice(sc * QC, (sc + 1) * QC)
                    dma(
                        XQ[:, :, ssl],
                        xqT[:, ssl].rearrange("(o p) s -> p o s", p=P),
                    )
                for sc in range(S_K // QC):
                    ssl = slice(sc * QC, (sc + 1) * QC)
                    dma(
                        XK[:, :, ssl],
                        xkT[:, ssl].rearrange("(o p) s -> p o s", p=P),
                    )

            def normalize(h, qc, po):
                hp, hr = h // 2, DK * (h % 2)
                qs = slice(qc * QC, (qc + 1) * QC)
                r = spool.tile([1, QC], F32, tag="r", name="r")
                nc.vector.reciprocal(r[:], po[DK:DK + 1, :])
                R = spool.tile([DK, QC], F32, tag="R", name="R")
                if MM_BCAST:
                    rps = psum.tile([DK, QC], F32, tag="s", bufs=2, name="rps")
                    nc.tensor.matmul(rps[:], ONES1[:, 0:DK], r[:],
                                     start=True, stop=True)
                    nc.vector.tensor_copy(R[:], rps[:])
                else:
                    nc.gpsimd.partition_broadcast(R[:], r[:])
                dest = QT[hr:hr + DK, hp, qs]
                nc.vector.tensor_tensor(dest, po[0:DK, :], R[:], MUL)
                nc.vector.tensor_tensor(
                    dest,
                    dest,
                    BIAS[hr:hr + DK, 8 + hp:9 + hp].to_broadcast((DK, QC)),
                    ADD,
                )

            def pair_item(hp, qc, sprinkles):
                """Scores+exp+attend for heads (2hp, 2hp+1) at q-chunk qc.

                B' kc-pairs feed ScalarE; D lags 2 kc-pairs behind; sprinkle
                units fill remaining PE slack.
                """
                qs = slice(qc * QC, (qc + 1) * QC)
                hA, hB = 2 * hp, 2 * hp + 1
                slabs = {}
                pos = {}
                for h in (hA, hB):
                    slabs[h] = apool.tile([P, NKC, QC], MDT, tag="slab",
                                          name="slab")
                    pos[h] = psum.tile([DK + 1, QC], F32, tag="o", bufs=2,
                                       name="ps_o")

                def d_pair(kc2):
                    for h in (hA, hB):
                        for kc in (2 * kc2, 2 * kc2 + 1):
                            nc.tensor.matmul(
                                pos[h][:],
                                VP[:, kc, h * (DK + 1):(h + 1) * (DK + 1)],
                                slabs[h][:, kc, :],
                                start=(kc == 0),
                                stop=(kc == NKC - 1),
                            )

                spr = list(sprinkles)
                for kc2 in range(NKC // 2):
                    ps2 = {}
                    for h in (hA, hB):
                        ps2[h] = psum.tile([P, 2 * QC], F32, tag="s", bufs=2,
                                           name="ps_s")
                    # the pair's matmuls issue back-to-back per kc so the two
                    # PE row groups (partitions 0-63 / 64-127) overlap
                    if NO_PACK:
                        emit_order = [(j, h) for h in (hA, hB) for j in range(2)]
                    else:
                        emit_order = [(j, h) for j in range(2) for h in (hA, hB)]
                    for j, h in emit_order:
                        kc = 2 * kc2 + j
                        hr = DK * (h % 2)
                        nc.tensor.matmul(
                            ps2[h][:, j * QC:(j + 1) * QC],
                            KT[hr:hr + DK, hp, kc * P:(kc + 1) * P],
                            QT[hr:hr + DK, hp, qs],
                            start=True,
                            stop=True,
                        )
                    for h in (hA, hB):
                        nc.scalar.activation(
                            slabs[h][:, 2 * kc2:2 * kc2 + 2, :].rearrange(
                                "p a b -> p (a b)"
                            ),
                            ps2[h][:],
                            EXP, scale=SCALE,
                        )
                    if spr:
                        spr.pop(0)()
                    if kc2 >= 2:
                        d_pair(kc2 - 2)
                for s in spr:  # any sprinkles that didn't fit the kc2 slots
                    s()
                d_pair(NKC // 2 - 2)
                d_pair(NKC // 2 - 1)
                for h in (hA, hB):
                    normalize(h, qc, pos[h])

            # ---------------- schedule ----------------
            vu = {kc4: v_chunk_units(kc4) for kc4 in range(8)}
            # preamble: V chunks 0-3, then Q/K load + head-pair-0 projections
            for kc4 in range(4):
                for u in vu[kc4]:
                    u()
            load_qk()
            for sc in range(NQC):
                proj_unit(XQ, wt_q, QT, 0, 0, sc)
            for sc in range(S_K // QC):
                proj_unit(XK, wt_k, KT, 4, 0, sc)

            def wo_unit():
                dma(WO[:], wo.rearrange("(o p) e -> p o e", p=P))

            def projs(hp):
                return (
                    [lambda sc=sc, hp=hp: proj_unit(XQ, wt_q, QT, 0, hp, sc)
                     for sc in range(NQC)]
                    + [lambda sc=sc, hp=hp: proj_unit(XK, wt_k, KT, 4, hp, sc)
                       for sc in range(S_K // QC)]
                )

            sprinkle_plan = {
                0: vu[4] + vu[5] + vu[6] + vu[7],      # (0,0): V chunks 4-7
                1: projs(1) + [wo_unit],               # (0,1)
                2: projs(2)[:3],                       # (1,0)
                3: projs(2)[3:],                       # (1,1)
                4: projs(3)[:3],                       # (2,0)
                5: projs(3)[3:],                       # (2,1)
                6: [],                                 # (3,0)
                7: [lambda ec=ec: f_unit(0, ec) for ec in range(8)],  # (3,1)
            }

            items = [(hp, qc) for hp in range(4) for qc in range(NQC)]

            for i, (hp, qc) in enumerate(items):
                pair_item(hp, qc, sprinkle_plan[i])
            for ec in range(8):
                f_unit(1, ec)


def _get_program():
    if "nc" not in _CACHE:
        _CACHE["nc"] = build_program()
    return _CACHE["nc"]


def make_in_maps(Q_decoder, K_encoder, V_encoder, Wq, bq, Wk, bk, Wv, bv, Wo):
    if MM_MODE == "bf16":
        import ml_dtypes
        mdt = np.dtype(ml_dtypes.bfloat16)
    else:
        mdt = np.dtype(np.float32)

    def mcast(x):
        return np.ascontiguousarray(np.asarray(x, dtype=np.float32).astype(mdt))

    def f32(x):
        return np.ascontiguousarray(np.asarray(x, dtype=np.float32))

    xT = {
        "xqT": [mcast(np.asarray(Q_decoder[b], np.float32).T) for b in range(B)],
        "xkT": [mcast(np.asarray(K_encoder[b], np.float32).T) for b in range(B)],
        "xvT": [mcast(np.asarray(V_encoder[b], np.float32).T) for b in range(B)],
    }
    Wq, Wk, Wv, Wo = (np.asarray(w, np.float32) for w in (Wq, Wk, Wv, Wo))
    bq, bk, bv = (np.asarray(v, np.float32) for v in (bq, bk, bv))

    in_maps = []
    for c in range(8):
        b, g = c // 2, c % 2
        cols = slice(F * g, F * (g + 1))
        in_maps.append({
            "xqT": xT["xqT"][b],
            "xkT": xT["xkT"][b],
            "xvT": xT["xvT"][b],
            "wq": mcast(Wq[:, cols]),
            "wk": mcast(Wk[:, cols]),
            "wv": mcast(Wv[:, cols]),
            "wo": mcast(Wo[cols, :]),
            "bq": f32(bq[cols]),
            "bk": f32(bk[cols]),
            "bv": f32(bv[cols]),
        })
    return in_maps


def gather(results, bo):
    bo = np.asarray(bo, dtype=np.float32)
    out = np.empty((B, S_Q, D), dtype=np.float32)
    for b in range(B):
        acc = results[2 * b]["fT"] + results[2 * b + 1]["fT"]
        out[b] = acc.T + bo
    return out


def kernel(**inputs) -> np.ndarray:
    nc = _get_program()
    in_maps = make_in_maps(**{k: v for k, v in inputs.items() if k != "bo"})
    res = run_bass_kernel_spmd(nc, in_maps, core_ids=list(range(8)))
    _CACHE["last_results"] = res
    return gather(res.results, inputs["bo"])

